# revision 26
# baseline (speedup 1.0000x reference)
"""NCE classifier scores kernel for Trainium2 (8 NeuronCores, SPMD).

scores = -(||q||^2 + ||p||^2 - 2 q.p) / T  for q = x[:8192], p = x[8192:].

Sharding: 2D data-parallel — 4 query shards x 2 proto shards. Core (r, c)
computes the [2048, 4096] slab out[r*2048:(r+1)*2048, c*4096:(c+1)*4096].

Proto staging is cooperative: each core loads only a 1024-row slice of its
proto half, transposes + casts it to the fp8 DoubleRow layout on chip, and
a 4-rank AllGather (cores sharing a proto half) distributes the packed
[128d, dg, h, p] tiles + centered half-norms. Per-core HBM input drops
from 24MB to 12MB and 3/4 of the proto transposes leave the PE.

Per-core device kernel (fp8 DoubleRow):
  - inputs are cast f32->bf16 during the load DMA; PE-identity transposes
    flip [row, d] tiles to [d, row]; ScalarE PSUM->SBUF copies cast to
    fp8e4m3 and pack the DoubleRow layout (contraction 256 per matmul).
  - matmuls run perf_mode=DoubleRow: 4 accumulating MMs of N=512 per
    (q-tile, column) — ~1.6x PE throughput vs bf16.
  - a single VectorE scalar_tensor_tensor applies both rank-1 corrections
    against centered half-norms (||.||^2/2 - 512), producing
    slab = q.p - qsq/2 - psq/2 + 1024 = -sq_dist/2 + 1024 (fp16).
  - the host applies the exact affine score = (slab - 1024) * 2/T during
    the f32 upcast/assembly (scores are linear in 1/T).
"""

import os
import sys

import numpy as np

NUM_BATCH = 8192
NUM_PROTO = 8192
DIM = 1024
N_CORES = 8
RSH = 4  # query shards
CSH = 2  # proto shards
QS = NUM_BATCH // RSH  # 2048 query rows per core
PS = NUM_PROTO // CSH  # 4096 proto rows per core (its half)
SS = PS // RSH  # 1024 staged proto rows per core
P = 128  # partitions
CH = 512  # proto chunk width (= one PSUM bank of f32)
NCH = PS // CH  # 8 columns
KT = DIM // P  # 8 contraction tiles (bf16 view)
DG = KT // 2  # 4 DoubleRow groups (256-wide contraction each)
NQT = QS // P  # 16 query tiles per core


def _install_axon_hooks_shim():
    """Provide antenv.axon_hooks (NTFF profiling hook) if the image lacks it."""
    try:
        import antenv.axon_hooks  # noqa: F401

        return
    except ImportError:
        pass
    import contextlib
    import ctypes
    import types

    mod = types.ModuleType("antenv.axon_hooks")
    _state = {"hook": None}
    mod.set_axon_ntff_profile_hook = lambda h: _state.__setitem__("hook", h)
    mod.get_axon_ntff_profile_hook = lambda: _state["hook"]
    sys.modules["antenv.axon_hooks"] = mod
    try:
        import antenv

        antenv.axon_hooks = mod
    except ImportError:
        pass
    so_path = "/opt/axon/libaxon_pjrt.so"
    if not os.path.exists(so_path):
        return
    try:
        lib = ctypes.CDLL(so_path)
        if not hasattr(lib, "axon_start_nrt_profile"):
            return
        lib.axon_start_nrt_profile.argtypes = [
            ctypes.POINTER(ctypes.c_int64),
            ctypes.c_size_t,
        ]
        lib.axon_start_nrt_profile.restype = ctypes.c_int64
        lib.axon_stop_nrt_profile.argtypes = [ctypes.c_char_p]
        lib.axon_stop_nrt_profile.restype = ctypes.c_int64

        @contextlib.contextmanager
        def _hook(output_dir, device_ids):
            import jax

            jax.devices()
            if device_ids:
                ids = (ctypes.c_int64 * len(device_ids))(*device_ids)
                rc = lib.axon_start_nrt_profile(ids, len(device_ids))
            else:
                rc = lib.axon_start_nrt_profile(None, 0)
            if rc != 0:
                raise RuntimeError(f"axon_start_nrt_profile rc={rc}")
            try:
                yield
            finally:
                n = lib.axon_stop_nrt_profile(str(output_dir).encode())
                print(f"profile: {n} file(s) written to {output_dir}")

        mod.set_axon_ntff_profile_hook(_hook)
    except OSError:
        pass


_NC_CACHE = {}


def _build_nc():
    if "nc" in _NC_CACHE:
        return _NC_CACHE["nc"]
    from contextlib import ExitStack

    import concourse.bacc as bacc
    import concourse.mybir as mybir
    import concourse.tile as tile
    from concourse.masks import make_identity

    F32 = mybir.dt.float32
    F32R = mybir.dt.float32r
    F16 = mybir.dt.float16
    BF16 = mybir.dt.bfloat16
    FP8 = mybir.dt.float8e4
    DR = mybir.MatmulPerfMode.DoubleRow
    SUB = mybir.AluOpType.subtract
    MULT = mybir.AluOpType.mult

    nc = bacc.Bacc("TRN2", target_bir_lowering=False, debug=False)
    xq = nc.dram_tensor("xq", [QS, DIM], F32, kind="ExternalInput").ap()
    xs = nc.dram_tensor("xs", [SS, DIM], F32, kind="ExternalInput").ap()
    xd = nc.dram_tensor("xd", [SS, DIM], F32, kind="ExternalInput").ap()
    out = nc.dram_tensor("out", [QS, PS], F16, kind="ExternalOutput").ap()
    # collective buffers: staged fp8 DoubleRow tiles + centered half-norms
    st_pt = nc.dram_tensor("st_pt", [P, DG * 2 * SS], FP8, kind="Internal")
    ag_pt = nc.dram_tensor("ag_pt", [RSH * P, DG * 2 * SS], FP8, kind="Internal")
    st_sq = nc.dram_tensor("st_sq", [1, SS], F32, kind="Internal")
    ag_sq = nc.dram_tensor("ag_sq", [RSH, SS], F32, kind="Internal")
    groups = [[0, 2, 4, 6], [1, 3, 5, 7]]  # cores sharing a proto half

    with tile.TileContext(nc) as tc:
        with ExitStack() as ctx:
            const = ctx.enter_context(tc.tile_pool(name="const", bufs=1))
            qpool = ctx.enter_context(tc.tile_pool(name="qpool", bufs=1))
            spool = ctx.enter_context(tc.tile_pool(name="spool", bufs=1))
            ptpool = ctx.enter_context(tc.tile_pool(name="ptpool", bufs=7))
            bpool = ctx.enter_context(tc.tile_pool(name="bpool", bufs=3))
            psqpool = ctx.enter_context(tc.tile_pool(name="psqpool", bufs=7))
            tpool = ctx.enter_context(tc.tile_pool(name="tpool", bufs=4))
            opool = ctx.enter_context(tc.tile_pool(name="opool", bufs=6))
            psum_mm = ctx.enter_context(
                tc.tile_pool(name="psum_mm", bufs=5, space="PSUM")
            )
            psum_tr = ctx.enter_context(
                tc.tile_pool(name="psum_tr", bufs=2, space="PSUM")
            )
            psum_bc = ctx.enter_context(
                tc.tile_pool(name="psum_bc", bufs=1, space="PSUM")
            )

            # ---- input DMA doorbells first; stage slice leads so the
            # AllGather can start as early as possible ----
            snat = spool.tile([P, SS // P, DIM], BF16)
            dnat = spool.tile([P, SS // P, DIM], BF16)
            qnat = qpool.tile([P, NQT, DIM], BF16)

            nc.gpsimd.dma_start(
                snat[:], xs[:].rearrange("(i p) d -> p i d", p=P)
            )

            def dma_q(h):  # 512 query rows (4 q-tiles)
                nc.gpsimd.dma_start(
                    qnat[:, h * 4 : (h + 1) * 4, :],
                    xq[h * 512 : (h + 1) * 512, :].rearrange(
                        "(i p) d -> p i d", p=P
                    ),
                )

            dma_q(0)
            dma_q(1)

            ident = const.tile([P, P], BF16)
            make_identity(nc, ident)
            ones_row_f = const.tile([1, P], F32)
            nc.gpsimd.memset(ones_row_f[:], 1.0)
            ones_row = ones_row_f.bitcast(F32R)

            nc.gpsimd.dma_start(
                dnat[:], xd[:].rearrange("(i p) d -> p i d", p=P)
            )
            dma_q(2)
            dma_q(3)

            # ---- stage: transpose own 1024 proto rows into fp8 DoubleRow
            # layout sg[dp, dg, h, p] (d = dg*256 + h*128 + dp), plus
            # centered half-norms; ship both through a 4-rank AllGather ----
            sg = spool.tile([P, DG, 2, SS], FP8)
            for k in range(KT):
                for g in range(2):  # two 512-row groups of the staged slice
                    pst = psum_tr.tile([P, CH], BF16, tag="pst")
                    for j in range(4):
                        nc.tensor.transpose(
                            pst[:, j * P : (j + 1) * P],
                            snat[:, g * 4 + j, k * P : (k + 1) * P],
                            ident[:],
                        )
                    nc.scalar.copy(
                        sg[:, k // 2, k % 2, g * CH : (g + 1) * CH], pst[:]
                    )
            psq8 = bpool.tile([P, SS // P], F32, tag="psq8")
            for i in range(SS // P):
                trash = tpool.tile([P, DIM], BF16, tag="trash")
                nc.scalar.activation(
                    out=trash[:],
                    in_=snat[:, i, :],
                    func=mybir.ActivationFunctionType.Square,
                    accum_out=psq8[:, i : i + 1],
                )
            psq8s = bpool.tile([P, SS // P], F32, tag="psq8s")
            nc.vector.tensor_scalar(psq8s[:], psq8[:], 0.5, 512.0, MULT, SUB)
            sq_row = bpool.tile([1, SS], F32, tag="sq_row")
            for i in range(SS // P):
                nc.sync.dma_start(
                    sq_row[:, i * P : (i + 1) * P], psq8s[:, i : i + 1]
                )
            nc.sync.dma_start(
                st_pt.ap()[:, :], sg[:].rearrange("p a b n -> p (a b n)")
            )
            nc.sync.dma_start(st_sq.ap()[:, :], sq_row[:])
            nc.gpsimd.collective_compute(
                "AllGather",
                mybir.AluOpType.bypass,
                replica_groups=groups,
                ins=[st_sq.ap()[:, :]],
                outs=[ag_sq.ap()[:, :]],
            )
            nc.gpsimd.collective_compute(
                "AllGather",
                mybir.AluOpType.bypass,
                replica_groups=groups,
                ins=[st_pt.ap()[:, :]],
                outs=[ag_pt.ap()[:, :]],
            )

            # ---- per-piece Q state: fp8 DoubleRow q-tiles + centered qsq ----
            qts = [[None] * DG for _ in range(4)]
            qsq_halves = [None] * 4

            def piece_tr(h):
                for dg in range(DG):
                    qt = qpool.tile([P, 2, 512], FP8, tag=f"qt{h}_{dg}")
                    qts[h][dg] = qt
                for k in range(KT):
                    pst = psum_tr.tile([P, CH], BF16, tag="pst")
                    for i in range(4):
                        nc.tensor.transpose(
                            pst[:, i * P : (i + 1) * P],
                            qnat[:, h * 4 + i, k * P : (k + 1) * P],
                            ident[:],
                        )
                    nc.scalar.copy(qts[h][k // 2][:, k % 2, :], pst[:])

            def piece_sq(h):
                qsq_raw = bpool.tile([P, 4], F32, tag="qsq_raw")
                for i in range(4):
                    trash = tpool.tile([P, DIM], BF16, tag="trash")
                    nc.scalar.activation(
                        out=trash[:],
                        in_=qnat[:, h * 4 + i, :],
                        func=mybir.ActivationFunctionType.Square,
                        accum_out=qsq_raw[:, i : i + 1],
                    )
                qsq_half = const.tile([P, 4], F32, tag=f"qsq_half{h}")
                nc.vector.tensor_scalar(
                    qsq_half[:], qsq_raw[:], 0.5, 512.0, MULT, SUB
                )
                qsq_halves[h] = qsq_half

            # ---- column prep ----
            pt_tiles = {}
            psq_b_tiles = {}
            dsq8s_box = [None]

            def lprep_sqs():
                # half-norms for the direct slice (columns 0-1)
                dsq8 = bpool.tile([P, SS // P], F32, tag="dsq8")
                for i in range(SS // P):
                    trash = tpool.tile([P, DIM], BF16, tag="trash")
                    nc.scalar.activation(
                        out=trash[:],
                        in_=dnat[:, i, :],
                        func=mybir.ActivationFunctionType.Square,
                        accum_out=dsq8[:, i : i + 1],
                    )
                dsq8s = bpool.tile([P, SS // P], F32, tag="dsq8s")
                nc.vector.tensor_scalar(dsq8s[:], dsq8[:], 0.5, 512.0, MULT, SUB)
                dsq8s_box[0] = dsq8s

            def lprep_tr(cc):
                # columns 0-1 are built locally from the direct slice while
                # the AllGather is in flight
                pta = ptpool.tile([P, DG, 2, CH], FP8, tag="pta")
                for k in range(KT):
                    pst = psum_tr.tile([P, CH], BF16, tag="pst")
                    for j in range(4):
                        nc.tensor.transpose(
                            pst[:, j * P : (j + 1) * P],
                            dnat[:, cc * 4 + j, k * P : (k + 1) * P],
                            ident[:],
                        )
                    nc.vector.tensor_copy(pta[:, k // 2, k % 2, :], pst[:])
                pt_tiles[cc] = pta

            def lprep_psq(cc):
                psq_row = bpool.tile([1, CH], F32, tag="psq_row")
                for j in range(4):
                    nc.sync.dma_start(
                        psq_row[:, j * P : (j + 1) * P],
                        dsq8s_box[0][:, cc * 4 + j : cc * 4 + j + 1],
                    )
                ps_b = psum_bc.tile([P, CH], F32, tag="ps_b")
                nc.tensor.matmul(
                    ps_b[:],
                    ones_row[:],
                    psq_row.bitcast(F32R)[:],
                    start=True,
                    stop=True,
                )
                psq_b = psqpool.tile([P, CH], F32, tag="psq_b")
                nc.vector.tensor_copy(psq_b[:], ps_b[:])
                psq_b_tiles[cc] = psq_b

            def col_prep(cc):
                rk = cc // 2  # rank (within the half) that staged this column
                pta = ptpool.tile([P, DG, 2, CH], FP8, tag="pta")
                nc.sync.dma_start(
                    pta[:],
                    ag_pt.ap()[rk * P : (rk + 1) * P, :]
                    .rearrange("p (a b s n) -> p a b s n", a=DG, b=2, s=2, n=CH)[
                        :, :, :, cc % 2, :
                    ],
                )
                psq_row = bpool.tile([1, CH], F32, tag="psq_row")
                nc.sync.dma_start(
                    psq_row[:],
                    ag_sq.ap()[rk : rk + 1, (cc % 2) * CH : (cc % 2 + 1) * CH],
                )
                ps_b = psum_bc.tile([P, CH], F32, tag="ps_b")
                nc.tensor.matmul(
                    ps_b[:],
                    ones_row[:],
                    psq_row.bitcast(F32R)[:],
                    start=True,
                    stop=True,
                )
                psq_b = psqpool.tile([P, CH], F32, tag="psq_b")
                nc.vector.tensor_copy(psq_b[:], ps_b[:])
                pt_tiles[cc] = pta
                psq_b_tiles[cc] = psq_b

            # ---- MM micro-block: 4 q-tiles (piece h) x one column ----
            ost_halves = {}

            def mb(h, cc):
                if h % 2 == 0:
                    ost = opool.tile([P, 8, CH], F16, tag="ost")
                    ost_halves[(cc, h // 2)] = ost
                else:
                    ost = ost_halves[(cc, h // 2)]
                for ql in range(4):
                    ps = psum_mm.tile([P, CH], F32, tag="mm")
                    for dg in range(DG):
                        nc.tensor.matmul(
                            ps[:],
                            qts[h][dg][:, :, ql * P : (ql + 1) * P],
                            pt_tiles[cc][:, dg, :, :],
                            start=(dg == 0),
                            stop=(dg == DG - 1),
                            perf_mode=DR,
                        )
                    nc.vector.scalar_tensor_tensor(
                        out=ost[:, (h % 2) * 4 + ql, :],
                        in0=ps[:],
                        scalar=qsq_halves[h][:, ql : ql + 1],
                        in1=psq_b_tiles[cc][:],
                        op0=SUB,
                        op1=SUB,
                    )
                if h % 2 == 1:  # half-column complete -> 1MB store
                    half = h // 2
                    dst = out[
                        half * 1024 : (half + 1) * 1024,
                        cc * CH : (cc + 1) * CH,
                    ].rearrange("(i p) n -> p i n", p=P)
                    if cc == NCH - 1:  # short tail DMAs at the very end
                        nc.sync.dma_start(dst[:, :4, :], ost[:, :4, :])
                        nc.sync.dma_start(dst[:, 4:, :], ost[:, 4:, :])
                    else:
                        nc.sync.dma_start(dst[:], ost[:])
                    ost_halves.pop((cc, half))

            # ---- schedule: local columns 0-1 + Q pieces overlap the
            # AllGather; gathered columns 2-7 stream afterwards ----
            piece_tr(0)
            piece_sq(0)
            piece_tr(1)
            piece_sq(1)
            lprep_tr(0)
            lprep_sqs()
            lprep_psq(0)
            lprep_tr(1)
            lprep_psq(1)
            mb(0, 0)
            mb(1, 0)
            piece_tr(2)
            piece_sq(2)
            mb(0, 1)
            mb(1, 1)
            mb(2, 0)
            piece_tr(3)
            piece_sq(3)
            mb(2, 1)
            mb(3, 0)
            mb(3, 1)
            for cc in range(2, NCH):
                col_prep(cc)
            # identity transposes keep the PE clock-gate warm across the
            # AllGather tail; they only run if the PE is otherwise idle here
            pstw = psum_tr.tile([P, CH], BF16, tag="pst")
            for i in range(40):
                nc.tensor.transpose(
                    pstw[:, (i % 4) * P : (i % 4 + 1) * P], ident[:], ident[:]
                )
            for cc in range(2, NCH):
                for h in range(4):
                    mb(h, cc)

    nc.compile()
    _NC_CACHE["nc"] = nc
    return nc


def _run(x, temperature, trace=False):
    _install_axon_hooks_shim()
    from concourse.bass_utils import run_bass_kernel_spmd

    nc = _build_nc()
    x = np.ascontiguousarray(np.asarray(x, dtype=np.float32))
    t = float(np.asarray(temperature, dtype=np.float32).reshape(()))
    in_maps = []
    for core in range(N_CORES):
        r, c = divmod(core, CSH)
        base = NUM_BATCH + c * PS + r * SS
        in_maps.append(
            {
                "xq": np.ascontiguousarray(x[r * QS : (r + 1) * QS]),
                "xs": np.ascontiguousarray(x[base : base + SS]),
                "xd": np.ascontiguousarray(
                    x[NUM_BATCH + c * PS : NUM_BATCH + c * PS + SS]
                ),
            }
        )
    res = run_bass_kernel_spmd(
        nc,
        in_maps,
        core_ids=list(range(N_CORES)),
        trace=trace,
        trace_cores=[0] if trace else None,
    )
    # device slab = q.p - qsq/2 - psq/2 + 1024 = -sq_dist/2 + 1024 (fp16);
    # scores = (slab - 1024) * 2/T, applied exactly in f32 on the host.
    scale = np.float32(2.0 / t)
    outf = np.empty((NUM_BATCH, NUM_PROTO), dtype=np.float32)
    for core in range(N_CORES):
        r, c = divmod(core, CSH)
        slab = np.asarray(res.results[core]["out"]).astype(np.float32)
        outf[r * QS : (r + 1) * QS, c * PS : (c + 1) * PS] = (
            slab - np.float32(1024.0)
        ) * scale
    return outf, res


def kernel(x, temperature, num_batch):
    assert int(num_batch) == NUM_BATCH, f"kernel hardcoded for num_batch={NUM_BATCH}"
    x = np.asarray(x)
    assert x.shape == (NUM_BATCH + NUM_PROTO, DIM), x.shape
    out, _ = _run(x, temperature, trace=False)
    return out


# revision 27
# speedup vs baseline: 1.0159x; 1.0159x over previous
"""NCE classifier scores kernel for Trainium2 (8 NeuronCores, SPMD).

scores = -(||q||^2 + ||p||^2 - 2 q.p) / T  for q = x[:8192], p = x[8192:].

Sharding: 2D data-parallel — 4 query shards x 2 proto shards. Core (r, c)
computes the [2048, 4096] slab out[r*2048:(r+1)*2048, c*4096:(c+1)*4096].

Proto staging is cooperative: each core loads only a 1024-row slice of its
proto half, transposes + casts it to the fp8 DoubleRow layout on chip, and
a 4-rank AllGather (cores sharing a proto half) distributes the packed
[128d, dg, h, p] tiles + centered half-norms. Per-core HBM input drops
from 24MB to 12MB and 3/4 of the proto transposes leave the PE.

Per-core device kernel (fp8 DoubleRow):
  - inputs are cast f32->bf16 during the load DMA; PE-identity transposes
    flip [row, d] tiles to [d, row]; ScalarE PSUM->SBUF copies cast to
    fp8e4m3 and pack the DoubleRow layout (contraction 256 per matmul).
  - matmuls run perf_mode=DoubleRow: 4 accumulating MMs of N=512 per
    (q-tile, column) — ~1.6x PE throughput vs bf16.
  - a single VectorE scalar_tensor_tensor applies both rank-1 corrections
    against centered half-norms (||.||^2/2 - 512), producing
    slab = q.p - qsq/2 - psq/2 + 1024 = -sq_dist/2 + 1024 (fp16).
  - the host applies the exact affine score = (slab - 1024) * 2/T during
    the f32 upcast/assembly (scores are linear in 1/T).
"""

import os
import sys

import numpy as np

NUM_BATCH = 8192
NUM_PROTO = 8192
DIM = 1024
N_CORES = 8
RSH = 4  # query shards
CSH = 2  # proto shards
QS = NUM_BATCH // RSH  # 2048 query rows per core
PS = NUM_PROTO // CSH  # 4096 proto rows per core (its half)
SS = PS // RSH  # 1024 staged proto rows per core
P = 128  # partitions
CH = 512  # proto chunk width (= one PSUM bank of f32)
NCH = PS // CH  # 8 columns
KT = DIM // P  # 8 contraction tiles (bf16 view)
DG = KT // 2  # 4 DoubleRow groups (256-wide contraction each)
NQT = QS // P  # 16 query tiles per core


def _install_axon_hooks_shim():
    """Provide antenv.axon_hooks (NTFF profiling hook) if the image lacks it."""
    try:
        import antenv.axon_hooks  # noqa: F401

        return
    except ImportError:
        pass
    import contextlib
    import ctypes
    import types

    mod = types.ModuleType("antenv.axon_hooks")
    _state = {"hook": None}
    mod.set_axon_ntff_profile_hook = lambda h: _state.__setitem__("hook", h)
    mod.get_axon_ntff_profile_hook = lambda: _state["hook"]
    sys.modules["antenv.axon_hooks"] = mod
    try:
        import antenv

        antenv.axon_hooks = mod
    except ImportError:
        pass
    so_path = "/opt/axon/libaxon_pjrt.so"
    if not os.path.exists(so_path):
        return
    try:
        lib = ctypes.CDLL(so_path)
        if not hasattr(lib, "axon_start_nrt_profile"):
            return
        lib.axon_start_nrt_profile.argtypes = [
            ctypes.POINTER(ctypes.c_int64),
            ctypes.c_size_t,
        ]
        lib.axon_start_nrt_profile.restype = ctypes.c_int64
        lib.axon_stop_nrt_profile.argtypes = [ctypes.c_char_p]
        lib.axon_stop_nrt_profile.restype = ctypes.c_int64

        @contextlib.contextmanager
        def _hook(output_dir, device_ids):
            import jax

            jax.devices()
            if device_ids:
                ids = (ctypes.c_int64 * len(device_ids))(*device_ids)
                rc = lib.axon_start_nrt_profile(ids, len(device_ids))
            else:
                rc = lib.axon_start_nrt_profile(None, 0)
            if rc != 0:
                raise RuntimeError(f"axon_start_nrt_profile rc={rc}")
            try:
                yield
            finally:
                n = lib.axon_stop_nrt_profile(str(output_dir).encode())
                print(f"profile: {n} file(s) written to {output_dir}")

        mod.set_axon_ntff_profile_hook(_hook)
    except OSError:
        pass


_NC_CACHE = {}


def _build_nc():
    if "nc" in _NC_CACHE:
        return _NC_CACHE["nc"]
    from contextlib import ExitStack

    import concourse.bacc as bacc
    import concourse.mybir as mybir
    import concourse.tile as tile
    from concourse.masks import make_identity

    F32 = mybir.dt.float32
    F32R = mybir.dt.float32r
    F16 = mybir.dt.float16
    BF16 = mybir.dt.bfloat16
    FP8 = mybir.dt.float8e4
    DR = mybir.MatmulPerfMode.DoubleRow
    SUB = mybir.AluOpType.subtract
    MULT = mybir.AluOpType.mult

    nc = bacc.Bacc("TRN2", target_bir_lowering=False, debug=False)
    xq = nc.dram_tensor("xq", [QS, DIM], F32, kind="ExternalInput").ap()
    xs = nc.dram_tensor("xs", [SS, DIM], F32, kind="ExternalInput").ap()
    xd = nc.dram_tensor("xd", [SS, DIM], F32, kind="ExternalInput").ap()
    out = nc.dram_tensor("out", [QS, PS], F16, kind="ExternalOutput").ap()
    # collective buffers: staged fp8 DoubleRow tiles + centered half-norms
    st_pt = nc.dram_tensor("st_pt", [P, DG * 2 * SS], FP8, kind="Internal")
    ag_pt = nc.dram_tensor("ag_pt", [RSH * P, DG * 2 * SS], FP8, kind="Internal")
    st_sq = nc.dram_tensor("st_sq", [1, SS], F32, kind="Internal")
    ag_sq = nc.dram_tensor("ag_sq", [RSH, SS], F32, kind="Internal")
    groups = [[0, 2, 4, 6], [1, 3, 5, 7]]  # cores sharing a proto half

    with tile.TileContext(nc) as tc:
        with ExitStack() as ctx:
            const = ctx.enter_context(tc.tile_pool(name="const", bufs=1))
            qpool = ctx.enter_context(tc.tile_pool(name="qpool", bufs=1))
            spool = ctx.enter_context(tc.tile_pool(name="spool", bufs=1))
            ptpool = ctx.enter_context(tc.tile_pool(name="ptpool", bufs=7))
            bpool = ctx.enter_context(tc.tile_pool(name="bpool", bufs=3))
            psqpool = ctx.enter_context(tc.tile_pool(name="psqpool", bufs=7))
            tpool = ctx.enter_context(tc.tile_pool(name="tpool", bufs=4))
            opool = ctx.enter_context(tc.tile_pool(name="opool", bufs=6))
            psum_mm = ctx.enter_context(
                tc.tile_pool(name="psum_mm", bufs=5, space="PSUM")
            )
            psum_tr = ctx.enter_context(
                tc.tile_pool(name="psum_tr", bufs=2, space="PSUM")
            )
            psum_bc = ctx.enter_context(
                tc.tile_pool(name="psum_bc", bufs=1, space="PSUM")
            )

            # ---- input DMA doorbells first; stage slice leads so the
            # AllGather can start as early as possible ----
            snat = spool.tile([P, SS // P, DIM], BF16)
            dnat = spool.tile([P, SS // P, DIM], BF16)
            qnat = qpool.tile([P, NQT, DIM], BF16)

            nc.gpsimd.dma_start(
                snat[:], xs[:].rearrange("(i p) d -> p i d", p=P)
            )
            nc.gpsimd.dma_start(
                dnat[:], xd[:].rearrange("(i p) d -> p i d", p=P)
            )

            def dma_q(h):  # 512 query rows (4 q-tiles)
                nc.gpsimd.dma_start(
                    qnat[:, h * 4 : (h + 1) * 4, :],
                    xq[h * 512 : (h + 1) * 512, :].rearrange(
                        "(i p) d -> p i d", p=P
                    ),
                )

            dma_q(0)
            dma_q(1)

            ident = const.tile([P, P], BF16)
            make_identity(nc, ident)
            ones_row_f = const.tile([1, P], F32)
            nc.gpsimd.memset(ones_row_f[:], 1.0)
            ones_row = ones_row_f.bitcast(F32R)

            dma_q(2)
            dma_q(3)

            # ---- stage: transpose own 1024 proto rows into fp8 DoubleRow
            # layout sg[dp, dg, h, p] (d = dg*256 + h*128 + dp), plus
            # centered half-norms; ship both through a 4-rank AllGather ----
            sg = spool.tile([P, DG, 2, SS], FP8)
            for k in range(KT):
                for g in range(2):  # two 512-row groups of the staged slice
                    pst = psum_tr.tile([P, CH], BF16, tag="pst")
                    for j in range(4):
                        nc.tensor.transpose(
                            pst[:, j * P : (j + 1) * P],
                            snat[:, g * 4 + j, k * P : (k + 1) * P],
                            ident[:],
                        )
                    nc.scalar.copy(
                        sg[:, k // 2, k % 2, g * CH : (g + 1) * CH], pst[:]
                    )
            psq8 = bpool.tile([P, SS // P], F32, tag="psq8")
            for i in range(SS // P):
                trash = tpool.tile([P, DIM], BF16, tag="trash")
                nc.scalar.activation(
                    out=trash[:],
                    in_=snat[:, i, :],
                    func=mybir.ActivationFunctionType.Square,
                    accum_out=psq8[:, i : i + 1],
                )
            psq8s = bpool.tile([P, SS // P], F32, tag="psq8s")
            nc.vector.tensor_scalar(psq8s[:], psq8[:], 0.5, 512.0, MULT, SUB)
            sq_row = bpool.tile([1, SS], F32, tag="sq_row")
            for i in range(SS // P):
                nc.sync.dma_start(
                    sq_row[:, i * P : (i + 1) * P], psq8s[:, i : i + 1]
                )
            nc.sync.dma_start(
                st_pt.ap()[:, :], sg[:].rearrange("p a b n -> p (a b n)")
            )
            nc.sync.dma_start(st_sq.ap()[:, :], sq_row[:])
            nc.gpsimd.collective_compute(
                "AllGather",
                mybir.AluOpType.bypass,
                replica_groups=groups,
                ins=[st_sq.ap()[:, :]],
                outs=[ag_sq.ap()[:, :]],
            )
            nc.gpsimd.collective_compute(
                "AllGather",
                mybir.AluOpType.bypass,
                replica_groups=groups,
                ins=[st_pt.ap()[:, :]],
                outs=[ag_pt.ap()[:, :]],
            )

            # ---- per-piece Q state: fp8 DoubleRow q-tiles + centered qsq ----
            qts = [[None] * DG for _ in range(4)]
            qsq_halves = [None] * 4

            def piece_tr(h):
                for dg in range(DG):
                    qt = qpool.tile([P, 2, 512], FP8, tag=f"qt{h}_{dg}")
                    qts[h][dg] = qt
                for k in range(KT):
                    pst = psum_tr.tile([P, CH], BF16, tag="pst")
                    for i in range(4):
                        nc.tensor.transpose(
                            pst[:, i * P : (i + 1) * P],
                            qnat[:, h * 4 + i, k * P : (k + 1) * P],
                            ident[:],
                        )
                    nc.scalar.copy(qts[h][k // 2][:, k % 2, :], pst[:])

            def piece_sq(h):
                qsq_raw = bpool.tile([P, 4], F32, tag="qsq_raw")
                for i in range(4):
                    trash = tpool.tile([P, DIM], BF16, tag="trash")
                    nc.scalar.activation(
                        out=trash[:],
                        in_=qnat[:, h * 4 + i, :],
                        func=mybir.ActivationFunctionType.Square,
                        accum_out=qsq_raw[:, i : i + 1],
                    )
                qsq_half = const.tile([P, 4], F32, tag=f"qsq_half{h}")
                nc.vector.tensor_scalar(
                    qsq_half[:], qsq_raw[:], 0.5, 512.0, MULT, SUB
                )
                qsq_halves[h] = qsq_half

            # ---- column prep ----
            pt_tiles = {}
            psq_b_tiles = {}
            dsq8s_box = [None]

            def lprep_sqs():
                # half-norms for the direct slice (columns 0-1)
                dsq8 = bpool.tile([P, SS // P], F32, tag="dsq8")
                for i in range(SS // P):
                    trash = tpool.tile([P, DIM], BF16, tag="trash")
                    nc.scalar.activation(
                        out=trash[:],
                        in_=dnat[:, i, :],
                        func=mybir.ActivationFunctionType.Square,
                        accum_out=dsq8[:, i : i + 1],
                    )
                dsq8s = bpool.tile([P, SS // P], F32, tag="dsq8s")
                nc.vector.tensor_scalar(dsq8s[:], dsq8[:], 0.5, 512.0, MULT, SUB)
                dsq8s_box[0] = dsq8s

            def lprep_tr(cc):
                # columns 0-1 are built locally from the direct slice while
                # the AllGather is in flight
                pta = ptpool.tile([P, DG, 2, CH], FP8, tag="pta")
                for k in range(KT):
                    pst = psum_tr.tile([P, CH], BF16, tag="pst")
                    for j in range(4):
                        nc.tensor.transpose(
                            pst[:, j * P : (j + 1) * P],
                            dnat[:, cc * 4 + j, k * P : (k + 1) * P],
                            ident[:],
                        )
                    nc.scalar.copy(pta[:, k // 2, k % 2, :], pst[:])
                pt_tiles[cc] = pta

            def lprep_psq(cc):
                psq_row = bpool.tile([1, CH], F32, tag="psq_row")
                for j in range(4):
                    nc.sync.dma_start(
                        psq_row[:, j * P : (j + 1) * P],
                        dsq8s_box[0][:, cc * 4 + j : cc * 4 + j + 1],
                    )
                ps_b = psum_bc.tile([P, CH], F32, tag="ps_b")
                nc.tensor.matmul(
                    ps_b[:],
                    ones_row[:],
                    psq_row.bitcast(F32R)[:],
                    start=True,
                    stop=True,
                )
                psq_b = psqpool.tile([P, CH], F32, tag="psq_b")
                nc.vector.tensor_copy(psq_b[:], ps_b[:])
                psq_b_tiles[cc] = psq_b

            def col_prep(cc):
                rk = cc // 2  # rank (within the half) that staged this column
                pta = ptpool.tile([P, DG, 2, CH], FP8, tag="pta")
                nc.sync.dma_start(
                    pta[:],
                    ag_pt.ap()[rk * P : (rk + 1) * P, :]
                    .rearrange("p (a b s n) -> p a b s n", a=DG, b=2, s=2, n=CH)[
                        :, :, :, cc % 2, :
                    ],
                )
                psq_row = bpool.tile([1, CH], F32, tag="psq_row")
                nc.sync.dma_start(
                    psq_row[:],
                    ag_sq.ap()[rk : rk + 1, (cc % 2) * CH : (cc % 2 + 1) * CH],
                )
                ps_b = psum_bc.tile([P, CH], F32, tag="ps_b")
                nc.tensor.matmul(
                    ps_b[:],
                    ones_row[:],
                    psq_row.bitcast(F32R)[:],
                    start=True,
                    stop=True,
                )
                psq_b = psqpool.tile([P, CH], F32, tag="psq_b")
                nc.vector.tensor_copy(psq_b[:], ps_b[:])
                pt_tiles[cc] = pta
                psq_b_tiles[cc] = psq_b

            # ---- MM micro-block: 4 q-tiles (piece h) x one column ----
            ost_halves = {}

            def mb(h, cc):
                if h % 2 == 0:
                    ost = opool.tile([P, 8, CH], F16, tag="ost")
                    ost_halves[(cc, h // 2)] = ost
                else:
                    ost = ost_halves[(cc, h // 2)]
                for ql in range(4):
                    ps = psum_mm.tile([P, CH], F32, tag="mm")
                    for dg in range(DG):
                        nc.tensor.matmul(
                            ps[:],
                            qts[h][dg][:, :, ql * P : (ql + 1) * P],
                            pt_tiles[cc][:, dg, :, :],
                            start=(dg == 0),
                            stop=(dg == DG - 1),
                            perf_mode=DR,
                        )
                    nc.vector.scalar_tensor_tensor(
                        out=ost[:, (h % 2) * 4 + ql, :],
                        in0=ps[:],
                        scalar=qsq_halves[h][:, ql : ql + 1],
                        in1=psq_b_tiles[cc][:],
                        op0=SUB,
                        op1=SUB,
                    )
                if h % 2 == 1:  # half-column complete -> 1MB store
                    half = h // 2
                    dst = out[
                        half * 1024 : (half + 1) * 1024,
                        cc * CH : (cc + 1) * CH,
                    ].rearrange("(i p) n -> p i n", p=P)
                    if cc == NCH - 1:  # short tail DMAs at the very end
                        nc.sync.dma_start(dst[:, :4, :], ost[:, :4, :])
                        nc.sync.dma_start(dst[:, 4:, :], ost[:, 4:, :])
                    else:
                        nc.sync.dma_start(dst[:], ost[:])
                    ost_halves.pop((cc, half))

            # ---- schedule: local columns 0-1 + Q pieces overlap the
            # AllGather; gathered columns 2-7 stream afterwards ----
            lprep_tr(0)
            piece_tr(0)
            lprep_sqs()
            lprep_psq(0)
            piece_sq(0)
            lprep_tr(1)
            lprep_psq(1)
            piece_tr(1)
            piece_sq(1)
            mb(0, 0)
            mb(1, 0)
            piece_tr(2)
            piece_sq(2)
            mb(0, 1)
            mb(1, 1)
            mb(2, 0)
            piece_tr(3)
            piece_sq(3)
            mb(2, 1)
            mb(3, 0)
            mb(3, 1)
            for cc in range(2, NCH):
                col_prep(cc)
            # identity transposes keep the PE clock-gate warm across the
            # AllGather tail; they only run if the PE is otherwise idle here
            pstw = psum_tr.tile([P, CH], BF16, tag="pst")
            for i in range(40):
                nc.tensor.transpose(
                    pstw[:, (i % 4) * P : (i % 4 + 1) * P], ident[:], ident[:]
                )
            for cc in range(2, NCH):
                for h in range(4):
                    mb(h, cc)

    nc.compile()
    _NC_CACHE["nc"] = nc
    return nc


def _run(x, temperature, trace=False):
    _install_axon_hooks_shim()
    from concourse.bass_utils import run_bass_kernel_spmd

    nc = _build_nc()
    x = np.ascontiguousarray(np.asarray(x, dtype=np.float32))
    t = float(np.asarray(temperature, dtype=np.float32).reshape(()))
    in_maps = []
    for core in range(N_CORES):
        r, c = divmod(core, CSH)
        base = NUM_BATCH + c * PS + r * SS
        in_maps.append(
            {
                "xq": np.ascontiguousarray(x[r * QS : (r + 1) * QS]),
                "xs": np.ascontiguousarray(x[base : base + SS]),
                "xd": np.ascontiguousarray(
                    x[NUM_BATCH + c * PS : NUM_BATCH + c * PS + SS]
                ),
            }
        )
    res = run_bass_kernel_spmd(
        nc,
        in_maps,
        core_ids=list(range(N_CORES)),
        trace=trace,
        trace_cores=[0] if trace else None,
    )
    # device slab = q.p - qsq/2 - psq/2 + 1024 = -sq_dist/2 + 1024 (fp16);
    # scores = (slab - 1024) * 2/T, applied exactly in f32 on the host.
    scale = np.float32(2.0 / t)
    outf = np.empty((NUM_BATCH, NUM_PROTO), dtype=np.float32)
    for core in range(N_CORES):
        r, c = divmod(core, CSH)
        slab = np.asarray(res.results[core]["out"]).astype(np.float32)
        outf[r * QS : (r + 1) * QS, c * PS : (c + 1) * PS] = (
            slab - np.float32(1024.0)
        ) * scale
    return outf, res


def kernel(x, temperature, num_batch):
    assert int(num_batch) == NUM_BATCH, f"kernel hardcoded for num_batch={NUM_BATCH}"
    x = np.asarray(x)
    assert x.shape == (NUM_BATCH + NUM_PROTO, DIM), x.shape
    out, _ = _run(x, temperature, trace=False)
    return out
